# revision 6
# baseline (speedup 1.0000x reference)
"""CRF negative-log-likelihood loss kernel for Trainium2 (8 NeuronCores).

Problem: B=256, S=2048, T=64 CRF loss (torchcrf-style), mask all-ones.

Strategy
--------
Data-parallel over batch: each of the 8 cores gets 32 batch rows.

Denominator (log-partition): forward/backward meet-in-the-middle.  The
forward chain  E_p = X_p * (W^T E_{p-1})  and the backward chain
C_s = X_s * (W C_{s+1})  (exp domain, X_s = exp(em_s - c0), W =
exp(trans)) advance together: one 128x128 block-diagonal matmul (top
block W, bottom block W^T as lhsT) + one [128,32] DVE multiply per
round.  1023 rounds instead of 2047.  Z = E_{S/2-1} . (W C_{S/2}).
A constant per-step prescale c0 plus exact renormalization every RN
rounds keeps fp32 in range; log shifts accumulate in c_acc and are
added back at the end:  den = ln(Zt) + c_f + c_b + S*c0.

Numerator (summed over the core's batch): one-hot matmuls.
  M = sum_{b,s} onehot_{b,s} x em_{b,s}      -> trace(M) = sum em[b,s,tag]
  N = sum_{b,s} onehot_{b,s} x onehot_{b,s+1} -> <N, trans> = sum trans[tag,tagnext]
The shifted one-hots come from a host-shifted tag array (pad -1 -> zero
row).  start/end transitions are folded into em rows s=0 / s=S-1 up
front, which also makes X_0 / X_{S-1} the correct chain initializers.

Per-core outputs: den[1,32], misc[1,2] = (em part incl start/end,
trans part).  Host: loss = -(sum_cores(misc.sum() - den.sum())) / B.
"""

import numpy as np

F32_NP = np.float32

B, S, T = 256, 2048, 64
NCORES = 8
BSH = B // NCORES  # 32
CHUNK = 128
C0 = 4.8204  # ~ ln(64 * e^0.5 * sinh(1)) : expected per-step log growth
RN = 24  # renorm every RN rounds

_NC_CACHE = {}


def build(n_chunks=16, bsh=BSH):
    """Build + compile the per-core Bass module. n_chunks*128 = seq len."""
    import concourse.bacc as bacc
    import concourse.mybir as mybir
    import concourse.tile as tile

    F32 = mybir.dt.float32
    AF = mybir.ActivationFunctionType
    ALU = mybir.AluOpType

    s_len = n_chunks * CHUNK
    half = n_chunks // 2
    assert half * 2 == n_chunks and half >= 1
    n_rounds = half * CHUNK - 1

    nc = bacc.Bacc("TRN2", target_bir_lowering=False, debug=False,
                   num_devices=NCORES)

    em_d = nc.dram_tensor("em", [bsh, s_len, T], F32, kind="ExternalInput")
    tags_d = nc.dram_tensor("tagsf", [bsh, s_len], F32, kind="ExternalInput")
    tagsq_d = nc.dram_tensor("tagsq", [bsh, s_len], F32, kind="ExternalInput")
    trans_d = nc.dram_tensor("trans", [T, T], F32, kind="ExternalInput")
    startm_d = nc.dram_tensor("startm", [128, T], F32, kind="ExternalInput")
    endm_d = nc.dram_tensor("endm", [128, T], F32, kind="ExternalInput")
    bones_d = nc.dram_tensor("bones", [128, 2], F32, kind="ExternalInput")
    bsel_d = nc.dram_tensor("bsel", [2, 128], F32, kind="ExternalInput")
    iota_d = nc.dram_tensor("iotat", [128, T], F32, kind="ExternalInput")
    ident_d = nc.dram_tensor("ident", [128, 128], F32, kind="ExternalInput")
    revj_d = nc.dram_tensor("revj", [128, 128], F32, kind="ExternalInput")
    den_d = nc.dram_tensor("den", [1, bsh], F32, kind="ExternalOutput")
    misc_d = nc.dram_tensor("misc", [1, 2], F32, kind="ExternalOutput")

    with tile.TileContext(nc) as tc:
        with (
            tc.tile_pool(name="consts", bufs=1) as consts,
            tc.tile_pool(name="xchunk", bufs=3) as xpool,
            tc.tile_pool(name="emt", bufs=3 * 2 * bsh) as empool,
            tc.tile_pool(name="ot", bufs=2 * bsh + 16) as opool,
            tc.tile_pool(name="state", bufs=4) as spool,
            tc.tile_pool(name="small", bufs=4) as smallpool,
            tc.tile_pool(name="pst", bufs=2, space="PSUM") as pstage,
            tc.tile_pool(name="pround", bufs=2, space="PSUM") as pround,
            tc.tile_pool(name="pacc", bufs=1, space="PSUM") as pacc,
            tc.tile_pool(name="pmisc", bufs=1, space="PSUM") as pmisc,
        ):
            # ---------------- constants / setup ----------------
            ident = consts.tile([128, 128], F32, tag="ident")
            nc.sync.dma_start(ident[:], ident_d.ap())
            revj = consts.tile([128, 128], F32, tag="revj")
            nc.sync.dma_start(revj[:], revj_d.ap())
            iota_t = consts.tile([128, T], F32, tag="iota")
            nc.sync.dma_start(iota_t[:], iota_d.ap())
            trans_sb = consts.tile([T, T], F32, tag="trans")
            nc.sync.dma_start(trans_sb[:], trans_d.ap())
            startm = consts.tile([128, T], F32, tag="startm")
            nc.sync.dma_start(startm[:], startm_d.ap())
            endm = consts.tile([128, T], F32, tag="endm")
            nc.sync.dma_start(endm[:], endm_d.ap())

            # block-diagonal lhsT: top-left W (for W^T @ E), bottom-right
            # W^T (for W @ C)
            blockw = consts.tile([128, 128], F32, tag="blockw")
            nc.vector.memset(blockw[:], 0.0)
            nc.scalar.activation(blockw[0:T, 0:T], trans_sb[:], AF.Exp)
            tp = pmisc.tile([128, 128], F32, tag="m128")
            nc.tensor.matmul(tp[0:T, 0:T], trans_sb[:], ident[0:T, 0:T],
                             start=True, stop=True)
            nc.scalar.activation(blockw[T:128, T:128], tp[0:T, 0:T], AF.Exp)

            blockones = consts.tile([128, 2], F32, tag="blockones")
            nc.sync.dma_start(blockones[:], bones_d.ap())
            blocksel = consts.tile([2, 128], F32, tag="blocksel")
            nc.sync.dma_start(blocksel[:], bsel_d.ap())
            ones64 = consts.tile([T, 1], F32, tag="ones64")
            nc.vector.memset(ones64[:], 1.0)
            ones2 = consts.tile([2, 1], F32, tag="ones2")
            nc.vector.memset(ones2[:], 1.0)
            negc0 = consts.tile([128, 1], F32, tag="negc0")
            nc.vector.memset(negc0[:], -C0)

            c_acc = consts.tile([2, bsh], F32, tag="cacc")
            nc.vector.memset(c_acc[:], 0.0)

            # tag tiles: tile[p, g] = tags[b, 128g + p]
            tag_tiles, tagq_tiles = [], []
            for b in range(bsh):
                tt = consts.tile([128, n_chunks], F32, tag=f"tags{b}")
                nc.sync.dma_start(
                    tt[:], tags_d.ap()[b].rearrange("(g p) -> p g", p=128))
                tag_tiles.append(tt)
                tq = consts.tile([128, n_chunks], F32, tag=f"tagsq{b}")
                nc.sync.dma_start(
                    tq[:], tagsq_d.ap()[b].rearrange("(g p) -> p g", p=128))
                tagq_tiles.append(tq)

            # numerator PSUM accumulators (alive whole kernel)
            m_ps = pacc.tile([T, T], F32, tag="m_ps")
            n_ps = pacc.tile([T, T], F32, tag="n_ps")

            num_state = {"m_first": True, "n_first": True,
                         "m_last": None, "n_last": None}

            def emit_chunk(c):
                """Produce X chunk c; also numerator matmuls for the two em
                chunks it consumes (gf ascending, gb descending)."""
                xc = xpool.tile([128, bsh, 128], F32, tag="xc")
                gf, gb = c, n_chunks - 1 - c
                em_tiles = {}
                for g in (gf, gb):
                    for b in range(bsh):
                        et = empool.tile([128, T], F32, tag="em")
                        nc.sync.dma_start(
                            et[:], em_d.ap()[b][g * 128:(g + 1) * 128, :])
                        if g == 0:
                            nc.gpsimd.tensor_add(et[:], et[:], startm[:])
                        if g == n_chunks - 1:
                            nc.gpsimd.tensor_add(et[:], et[:], endm[:])
                        em_tiles[(g, b)] = et

                for g in (gf, gb):
                    for b in range(bsh):
                        ot = opool.tile([128, T], F32, tag="o")
                        nc.gpsimd.tensor_scalar(
                            ot[:], iota_t[:], tag_tiles[b][:, g:g + 1], None,
                            op0=ALU.is_equal)
                        oq = opool.tile([128, T], F32, tag="oq")
                        nc.gpsimd.tensor_scalar(
                            oq[:], iota_t[:], tagq_tiles[b][:, g:g + 1], None,
                            op0=ALU.is_equal)
                        mm = nc.tensor.matmul(
                            m_ps[:], ot[:], em_tiles[(g, b)][:],
                            start=num_state["m_first"], stop=False,
                            skip_group_check=True)
                        num_state["m_first"] = False
                        num_state["m_last"] = mm
                        nn_ = nc.tensor.matmul(
                            n_ps[:], ot[:], oq[:],
                            start=num_state["n_first"], stop=False,
                            skip_group_check=True)
                        num_state["n_first"] = False
                        num_state["n_last"] = nn_

                # transpose (via matmul with I / flipped J) + exp into X
                for a in range(bsh // 4):
                    st = pstage.tile([128, 4, 128], F32, tag="stage")
                    for j in range(4):
                        b = 4 * a + j
                        nc.tensor.matmul(st[0:T, j, :], em_tiles[(gf, b)][:],
                                         ident[:], start=True, stop=True)
                        nc.tensor.matmul(st[T:128, j, :], em_tiles[(gb, b)][:],
                                         revj[:], start=True, stop=True)
                    nc.scalar.activation(xc[:, 4 * a:4 * a + 4, :], st[:],
                                         AF.Exp, bias=negc0[:])
                return xc

            # ---------------- main pipeline ----------------
            xchunks = {0: emit_chunk(0)}
            if half > 1:
                xchunks[1] = emit_chunk(1)

            state = spool.tile([128, bsh], F32, tag="st")
            nc.gpsimd.tensor_copy(state[:], xchunks[0][:, :, 0])

            for r in range(1, n_rounds + 1):
                c, j = r >> 7, r & 127
                if c + 2 <= half - 1 and j == 1:
                    xchunks[c + 2] = emit_chunk(c + 2)
                    xchunks.pop(c - 1, None)
                p = pround.tile([128, bsh], F32, tag="p")
                nc.tensor.matmul(p[:], blockw[:], state[:],
                                 start=True, stop=True)
                state = spool.tile([128, bsh], F32, tag="st")
                nc.vector.tensor_mul(state[:], p[:], xchunks[c][:, :, j])

                if r % RN == 0 and r < n_rounds:
                    mass = pmisc.tile([2, bsh], F32, tag="m2")
                    nc.tensor.matmul(mass[:], blockones[:], state[:],
                                     start=True, stop=True)
                    rmass = smallpool.tile([2, bsh], F32, tag="rm")
                    nc.vector.reciprocal(rmass[:], mass[:])
                    lnm = smallpool.tile([2, bsh], F32, tag="lnm")
                    nc.scalar.activation(lnm[:], mass[:], AF.Ln)
                    nc.gpsimd.tensor_add(c_acc[:], c_acc[:], lnm[:])
                    rbc = pmisc.tile([128, 128], F32, tag="m128")
                    nc.tensor.matmul(rbc[:, 0:bsh], blocksel[:], rmass[:],
                                     start=True, stop=True)
                    nstate = spool.tile([128, bsh], F32, tag="st")
                    nc.vector.tensor_mul(nstate[:], state[:], rbc[:, 0:bsh])
                    state = nstate

            # ---------------- final combine ----------------
            # beta = W @ C on partitions 0..63 (aligned base-64 matmul)
            pf = pround.tile([128, bsh], F32, tag="p")
            nc.tensor.matmul(pf[0:T, :], blockw[T:128, T:128],
                             state[T:128, :], start=True, stop=True)
            y = smallpool.tile([T, bsh], F32, tag="y")
            nc.vector.tensor_mul(y[:], state[0:T, :], pf[0:T, :])
            z = pmisc.tile([2, bsh], F32, tag="m2")
            nc.tensor.matmul(z[0:1, :], ones64[:], y[:], start=True, stop=True)
            den_sb = smallpool.tile([1, bsh], F32, tag="densb")
            nc.scalar.activation(den_sb[:], z[0:1, :], AF.Ln)
            csum = pmisc.tile([2, bsh], F32, tag="m2")
            nc.tensor.matmul(csum[0:1, :], ones2[:], c_acc[:],
                             start=True, stop=True)
            csum_sb = smallpool.tile([1, bsh], F32, tag="csum")
            nc.scalar.activation(csum_sb[:], csum[0:1, :], AF.Copy)
            nc.gpsimd.tensor_add(den_sb[:], den_sb[:], csum_sb[:])
            nc.gpsimd.tensor_scalar_add(den_sb[:], den_sb[:],
                                        float(s_len) * C0)
            nc.sync.dma_start(den_d.ap(), den_sb[:])

            # numerator finish
            num_state["m_last"].ins.stop_tensor_calc = True
            num_state["n_last"].ins.stop_tensor_calc = True
            scr = smallpool.tile([T, T], F32, tag="scr")
            acc2 = smallpool.tile([T, 2], F32, tag="acc2")
            nc.vector.scalar_tensor_tensor(
                scr[:], ident[0:T, 0:T], 1.0, m_ps[:],
                op0=ALU.bypass, op1=ALU.mult, accum_out=acc2[:, 0:1])
            scr2 = smallpool.tile([T, T], F32, tag="scr2")
            nc.vector.scalar_tensor_tensor(
                scr2[:], trans_sb[:], 1.0, n_ps[:],
                op0=ALU.bypass, op1=ALU.mult, accum_out=acc2[:, 1:2])
            misc_ps = pmisc.tile([2, bsh], F32, tag="m2")
            nc.tensor.matmul(misc_ps[0:1, 0:2], ones64[:], acc2[:],
                             start=True, stop=True)
            misc_sb = smallpool.tile([1, 2], F32, tag="miscsb")
            nc.scalar.activation(misc_sb[:], misc_ps[0:1, 0:2], AF.Copy)
            nc.sync.dma_start(misc_d.ap(), misc_sb[:])

    nc.compile()
    return nc


def _get_nc(n_chunks=16, bsh=BSH):
    key = (n_chunks, bsh)
    if key not in _NC_CACHE:
        _NC_CACHE[key] = build(n_chunks, bsh)
    return _NC_CACHE[key]


def _consts(startr, endr):
    iota = np.broadcast_to(np.arange(T, dtype=F32_NP), (128, T)).copy()
    ident = np.eye(128, dtype=F32_NP)
    revj = np.eye(128, dtype=F32_NP)[::-1].copy()
    startm = np.zeros((128, T), dtype=F32_NP)
    startm[0] = startr
    endm = np.zeros((128, T), dtype=F32_NP)
    endm[127] = endr
    bones = np.zeros((128, 2), dtype=F32_NP)
    bones[0:T, 0] = 1.0
    bones[T:128, 1] = 1.0
    bsel = np.zeros((2, 128), dtype=F32_NP)
    bsel[0, 0:T] = 1.0
    bsel[1, T:128] = 1.0
    return iota, ident, revj, startm, endm, bones, bsel


def _shift_tags(tags_f):
    tq = np.empty_like(tags_f)
    tq[:, :-1] = tags_f[:, 1:]
    tq[:, -1] = -1.0
    return tq


def kernel(emissions, start_transitions, end_transitions, transitions,
           tags, mask):
    """Full-input entry point; shards over 8 NeuronCores internally."""
    from concourse.bass_utils import run_bass_kernel_spmd

    emissions = np.ascontiguousarray(np.asarray(emissions, dtype=F32_NP))
    tags_np = np.asarray(tags)
    mask_np = np.asarray(mask)
    assert emissions.shape == (B, S, T)
    assert (np.asarray(mask_np) != 0).all(), "kernel assumes all-ones mask"
    trans = np.asarray(transitions, dtype=F32_NP).reshape(T, T)
    startr = np.asarray(start_transitions, dtype=F32_NP).ravel()
    endr = np.asarray(end_transitions, dtype=F32_NP).ravel()
    tags_f = np.ascontiguousarray(tags_np.astype(F32_NP).reshape(B, S))
    tags_q = _shift_tags(tags_f)

    iota, ident, revj, startm, endm, bones, bsel = _consts(startr, endr)
    nc = _get_nc()

    in_maps = []
    for cidx in range(NCORES):
        sl = slice(cidx * BSH, (cidx + 1) * BSH)
        in_maps.append({
            "em": emissions[sl],
            "tagsf": tags_f[sl],
            "tagsq": tags_q[sl],
            "trans": trans,
            "startm": startm,
            "endm": endm,
            "bones": bones,
            "bsel": bsel,
            "iotat": iota,
            "ident": ident,
            "revj": revj,
        })
    res = run_bass_kernel_spmd(nc, in_maps, core_ids=list(range(NCORES)))

    num_total = 0.0
    den_total = 0.0
    for cidx in range(NCORES):
        r = res.results[cidx]
        num_total += float(r["misc"].sum())
        den_total += float(r["den"].sum())
    loss = -(num_total - den_total) / float(B)
    return np.float32(loss)


# revision 14
# speedup vs baseline: 5.0488x; 5.0488x over previous
"""CRF negative-log-likelihood loss kernel for Trainium2 (8 NeuronCores).

Problem: B=256, S=2048, T=64 CRF loss (torchcrf-style), mask all-ones.

Strategy
--------
Data-parallel over batch: each of the 8 cores gets 32 batch rows.

Denominator (log-partition): forward/backward meet-in-the-middle.  The
forward chain  E_p = X_p * (W^T E_{p-1})  and the backward chain
C_s = X_s * (W C_{s+1})  (exp domain, X_s = exp(em_s - c0), W =
exp(trans)) advance together: one 128x128 block-diagonal matmul (top
block W, bottom block W^T as lhsT, bf16) + one [128,32] DVE multiply
per round.  1023 rounds instead of 2047.  Z = E_{S/2-1} . (W C_{S/2}).
A constant per-step prescale c0 plus exact renormalization every RN
rounds keeps fp32 in range; the actually-applied bf16 reciprocals are
logged exactly (c_acc -= ln(rhat)) and added back at the end:
  den = ln(Zt) + c_f + c_b + S*c0.

Numerator (summed over the core's batch): one-hot matmuls, all bf16
(one-hots are exact in bf16; em is bf16-rounded, error ~1e-6 relative).
  M = sum_{b,s} onehot_{b,s} x em_{b,s}      -> trace(M) = sum em[b,s,tag]
  N = sum_{b,s} onehot_{b,s} x onehot_{b,s+1} -> <N, trans> = sum trans[tag,tagnext]
Shifted one-hots come from a host-shifted tag array (pad -1 -> zero
row).  start/end transitions are folded into em rows s=0 / s=S-1 on the
host, which also makes X_0 / X_{S-1} the correct chain initializers.

Emissions travel as bf16 (half the DMA bytes); exp() output X stays
f32.  X chunks are stored j-major ([128, j, b]) so the per-round DVE
read is contiguous.

Per-core outputs: den[1,32] f32, misc[1,2] f32 = (em part incl
start/end, trans part).  Host: loss = -(sum(misc) - sum(den)) / B.
"""

import contextlib

import numpy as np
import ml_dtypes

F32_NP = np.float32
BF16_NP = ml_dtypes.bfloat16

B, S, T = 256, 2048, 64
NCORES = 8
BSH = B // NCORES  # 32
CHUNK = 128
C0 = 4.8204  # ~ ln(64 * e^0.5 * sinh(1)) : expected per-step log growth
RN = 24  # renorm every RN rounds

_NC_CACHE = {}


def build(n_chunks=16, bsh=BSH, nrep=1, fake_x=False, no_num=False,
          no_rounds=False, rn=RN, pround_bufs=2, spool_bufs=6):
    """Build + compile the per-core Bass module. n_chunks*128 = seq len.

    nrep>1 wraps the whole computation in a device-side loop (timing
    only); fake_x / no_num / no_rounds strip parts for cost bisection."""
    import concourse.bacc as bacc
    import concourse.mybir as mybir
    import concourse.tile as tile

    F32 = mybir.dt.float32
    BF16 = mybir.dt.bfloat16
    AF = mybir.ActivationFunctionType
    ALU = mybir.AluOpType

    s_len = n_chunks * CHUNK
    half = n_chunks // 2
    assert half * 2 == n_chunks and half >= 1
    n_rounds = half * CHUNK - 1

    nc = bacc.Bacc("TRN2", target_bir_lowering=False, debug=False,
                   num_devices=NCORES)

    em_d = nc.dram_tensor("em", [bsh, s_len, T], BF16, kind="ExternalInput")
    tags_d = nc.dram_tensor("tagsf", [bsh, s_len], F32, kind="ExternalInput")
    tagsq_d = nc.dram_tensor("tagsq", [bsh, s_len], F32, kind="ExternalInput")
    trans_d = nc.dram_tensor("trans", [T, T], F32, kind="ExternalInput")
    bones_d = nc.dram_tensor("bones", [128, 2], BF16, kind="ExternalInput")
    bsel_d = nc.dram_tensor("bsel", [2, 128], BF16, kind="ExternalInput")
    iota_d = nc.dram_tensor("iotat", [128, T], BF16, kind="ExternalInput")
    ident_d = nc.dram_tensor("ident", [128, 128], F32, kind="ExternalInput")
    identb_d = nc.dram_tensor("identb", [128, 128], BF16,
                              kind="ExternalInput")
    revjb_d = nc.dram_tensor("revjb", [128, 128], BF16, kind="ExternalInput")
    den_d = nc.dram_tensor("den", [1, bsh], F32, kind="ExternalOutput")
    misc_d = nc.dram_tensor("misc", [1, 2], F32, kind="ExternalOutput")

    with tile.TileContext(nc) as tc, nc.allow_low_precision(
            reason="bf16 state/weights validated against f64 reference"):
        with (
            tc.tile_pool(name="consts", bufs=1) as consts,
            tc.tile_pool(name="xchunk", bufs=3) as xpool,
            tc.tile_pool(name="emt", bufs=6) as empool,
            tc.tile_pool(name="ot", bufs=2 * bsh + 8) as opool,
            tc.tile_pool(name="state", bufs=spool_bufs) as spool,
            tc.tile_pool(name="small", bufs=4) as smallpool,
            tc.tile_pool(name="pst", bufs=2, space="PSUM") as pstage,
            tc.tile_pool(name="pround", bufs=pround_bufs,
                         space="PSUM") as pround,
            tc.tile_pool(name="pacc", bufs=1, space="PSUM") as pacc,
            tc.tile_pool(name="pmisc", bufs=1, space="PSUM") as pmisc,
        ):
            rep_ctx = (tc.For_i(0, nrep, 1) if nrep > 1
                       else contextlib.nullcontext())
            with rep_ctx:
                # ---------------- constants / setup ----------------
                ident = consts.tile([128, 128], F32, tag="ident")
                nc.sync.dma_start(ident[:], ident_d.ap())
                identb = consts.tile([128, 128], BF16, tag="identb")
                nc.sync.dma_start(identb[:], identb_d.ap())
                revjb = consts.tile([128, 128], BF16, tag="revjb")
                nc.sync.dma_start(revjb[:], revjb_d.ap())
                iota_t = consts.tile([128, T], BF16, tag="iota")
                nc.sync.dma_start(iota_t[:], iota_d.ap())
                trans_sb = consts.tile([T, T], F32, tag="trans")
                nc.sync.dma_start(trans_sb[:], trans_d.ap())

                # block-diagonal lhsT (bf16): top-left W (for W^T @ E),
                # bottom-right W^T (for W @ C)
                blockw = consts.tile([128, 128], BF16, tag="blockw")
                nc.vector.memset(blockw[:], 0.0)
                nc.scalar.activation(blockw[0:T, 0:T], trans_sb[:], AF.Exp)
                tp = pmisc.tile([128, 128], F32, tag="m128")
                nc.tensor.matmul(tp[0:T, 0:T], trans_sb[:], ident[0:T, 0:T],
                                 start=True, stop=True)
                nc.scalar.activation(blockw[T:128, T:128], tp[0:T, 0:T],
                                     AF.Exp)

                blockones = consts.tile([128, 2], BF16, tag="blockones")
                nc.sync.dma_start(blockones[:], bones_d.ap())
                blocksel = consts.tile([2, 128], BF16, tag="blocksel")
                nc.sync.dma_start(blocksel[:], bsel_d.ap())
                ones64 = consts.tile([T, 1], F32, tag="ones64")
                nc.vector.memset(ones64[:], 1.0)
                ones2 = consts.tile([2, 1], F32, tag="ones2")
                nc.vector.memset(ones2[:], 1.0)
                negc0 = consts.tile([128, 1], F32, tag="negc0")
                nc.vector.memset(negc0[:], -C0)

                c_acc = consts.tile([2, bsh], F32, tag="cacc")
                nc.vector.memset(c_acc[:], 0.0)

                # tag tiles: tile[p, g] = tags[b, 128g + p]
                tag_tiles, tagq_tiles = [], []
                for b in range(bsh):
                    tt = consts.tile([128, n_chunks], F32, tag=f"tags{b}")
                    nc.sync.dma_start(
                        tt[:],
                        tags_d.ap()[b].rearrange("(g p) -> p g", p=128))
                    tag_tiles.append(tt)
                    tq = consts.tile([128, n_chunks], F32, tag=f"tagsq{b}")
                    nc.sync.dma_start(
                        tq[:],
                        tagsq_d.ap()[b].rearrange("(g p) -> p g", p=128))
                    tagq_tiles.append(tq)

                # numerator PSUM accumulators (alive whole kernel)
                m_ps = pacc.tile([T, T], F32, tag="m_ps")
                n_ps = pacc.tile([T, T], F32, tag="n_ps")

                num_state = {"m_first": True, "n_first": True,
                             "m_last": None, "n_last": None}

                def emit_chunk(c):
                    """Produce X chunk c ([128, j, b], f32); plus numerator
                    matmuls for the two em chunks it consumes."""
                    xc = xpool.tile([128, 128, bsh], F32, tag="xc")
                    if fake_x:
                        nc.gpsimd.memset(xc[:], 0.0133)
                        return xc
                    gf, gb = c, n_chunks - 1 - c
                    emg = {}
                    for g in (gf, gb):
                        eg = empool.tile([128, bsh, T], BF16, tag="em")
                        nc.sync.dma_start(
                            eg[:],
                            em_d.ap()[:, g * 128:(g + 1) * 128, :]
                            .rearrange("b s t -> s b t"))
                        emg[g] = eg

                    if not no_num:
                        for g in (gf, gb):
                            for b in range(bsh):
                                ot = opool.tile([128, T], BF16, tag="o")
                                nc.gpsimd.tensor_scalar(
                                    ot[:], iota_t[:],
                                    tag_tiles[b][:, g:g + 1], None,
                                    op0=ALU.is_equal)
                                oq = opool.tile([128, T], BF16, tag="oq")
                                nc.gpsimd.tensor_scalar(
                                    oq[:], iota_t[:],
                                    tagq_tiles[b][:, g:g + 1], None,
                                    op0=ALU.is_equal)
                                mm = nc.tensor.matmul(
                                    m_ps[:], ot[:], emg[g][:, b, :],
                                    start=num_state["m_first"], stop=False,
                                    skip_group_check=True)
                                num_state["m_first"] = False
                                num_state["m_last"] = mm
                                nn_ = nc.tensor.matmul(
                                    n_ps[:], ot[:], oq[:],
                                    start=num_state["n_first"], stop=False,
                                    skip_group_check=True)
                                num_state["n_first"] = False
                                num_state["n_last"] = nn_

                    # transpose (matmul with I / flipped J) + exp into X;
                    # X is written j-major: xc[:, j, b]
                    for a in range(bsh // 4):
                        st = pstage.tile([128, 4, 128], F32, tag="stage")
                        for j in range(4):
                            b = 4 * a + j
                            nc.tensor.matmul(st[0:T, j, :],
                                             emg[gf][:, b, :], identb[:],
                                             start=True, stop=True)
                            nc.tensor.matmul(st[T:128, j, :],
                                             emg[gb][:, b, :], revjb[:],
                                             start=True, stop=True)
                        nc.scalar.activation(
                            xc[:, :, 4 * a:4 * a + 4],
                            st[:].rearrange("p b j -> p j b"),
                            AF.Exp, bias=negc0[:])
                    return xc

                # ---------------- main pipeline ----------------
                xchunks = {0: emit_chunk(0)}
                if half > 1:
                    xchunks[1] = emit_chunk(1)

                state = spool.tile([128, bsh], BF16, tag="st")
                nc.gpsimd.tensor_copy(state[:], xchunks[0][:, 0, :])

                r_end = 0 if no_rounds else n_rounds
                for r in range(1, r_end + 1):
                    c, j = r >> 7, r & 127
                    if c + 2 <= half - 1 and j == 1:
                        xchunks[c + 2] = emit_chunk(c + 2)
                        xchunks.pop(c - 1, None)
                    p = pround.tile([128, bsh], F32, tag="p")
                    nc.tensor.matmul(p[:], blockw[:], state[:],
                                     start=True, stop=True)
                    state = spool.tile([128, bsh], BF16, tag="st")
                    nc.vector.tensor_mul(state[:], p[:], xchunks[c][:, j, :])

                    if r % rn == 0 and r < n_rounds:
                        mass = pmisc.tile([2, bsh], F32, tag="m2")
                        nc.tensor.matmul(mass[:], blockones[:], state[:],
                                         start=True, stop=True)
                        rmass = smallpool.tile([2, bsh], BF16, tag="rm")
                        nc.vector.reciprocal(rmass[:], mass[:])
                        lnr = smallpool.tile([2, bsh], F32, tag="lnr")
                        nc.scalar.activation(lnr[:], rmass[:], AF.Ln)
                        nc.gpsimd.tensor_sub(c_acc[:], c_acc[:], lnr[:])
                        rbc = pmisc.tile([128, 128], F32, tag="m128")
                        nc.tensor.matmul(rbc[:, 0:bsh], blocksel[:],
                                         rmass[:], start=True, stop=True)
                        nstate = spool.tile([128, bsh], BF16, tag="st")
                        nc.vector.tensor_mul(nstate[:], state[:],
                                             rbc[:, 0:bsh])
                        state = nstate

                # ---------------- final combine ----------------
                # beta = W @ C on partitions 0..63 (aligned base-64 matmul)
                pf = pround.tile([128, bsh], F32, tag="p")
                nc.tensor.matmul(pf[0:T, :], blockw[T:128, T:128],
                                 state[T:128, :], start=True, stop=True)
                y = smallpool.tile([T, bsh], F32, tag="y")
                nc.vector.tensor_mul(y[:], state[0:T, :], pf[0:T, :])
                z = pmisc.tile([2, bsh], F32, tag="m2")
                nc.tensor.matmul(z[0:1, :], ones64[:], y[:],
                                 start=True, stop=True)
                den_sb = smallpool.tile([1, bsh], F32, tag="densb")
                nc.scalar.activation(den_sb[:], z[0:1, :], AF.Ln)
                csum = pmisc.tile([2, bsh], F32, tag="m2")
                nc.tensor.matmul(csum[0:1, :], ones2[:], c_acc[:],
                                 start=True, stop=True)
                csum_sb = smallpool.tile([1, bsh], F32, tag="csum")
                nc.scalar.activation(csum_sb[:], csum[0:1, :], AF.Copy)
                nc.gpsimd.tensor_add(den_sb[:], den_sb[:], csum_sb[:])
                nc.gpsimd.tensor_scalar_add(den_sb[:], den_sb[:],
                                            float(s_len) * C0)
                nc.sync.dma_start(den_d.ap(), den_sb[:])

                # numerator finish
                if num_state["m_last"] is None:
                    misc_sbz = smallpool.tile([1, 2], F32, tag="miscsb")
                    nc.vector.memset(misc_sbz[:], 0.0)
                    nc.sync.dma_start(misc_d.ap(), misc_sbz[:])
                else:
                    num_state["m_last"].ins.stop_tensor_calc = True
                    num_state["n_last"].ins.stop_tensor_calc = True
                    scr = smallpool.tile([T, T], F32, tag="scr")
                    acc2 = smallpool.tile([T, 2], F32, tag="acc2")
                    nc.vector.scalar_tensor_tensor(
                        scr[:], ident[0:T, 0:T], 1.0, m_ps[:],
                        op0=ALU.bypass, op1=ALU.mult, accum_out=acc2[:, 0:1])
                    scr2 = smallpool.tile([T, T], F32, tag="scr2")
                    nc.vector.scalar_tensor_tensor(
                        scr2[:], trans_sb[:], 1.0, n_ps[:],
                        op0=ALU.bypass, op1=ALU.mult, accum_out=acc2[:, 1:2])
                    misc_ps = pmisc.tile([2, bsh], F32, tag="m2")
                    nc.tensor.matmul(misc_ps[0:1, 0:2], ones64[:], acc2[:],
                                     start=True, stop=True)
                    misc_sb = smallpool.tile([1, 2], F32, tag="miscsb")
                    nc.scalar.activation(misc_sb[:], misc_ps[0:1, 0:2],
                                         AF.Copy)
                    nc.sync.dma_start(misc_d.ap(), misc_sb[:])

    nc.compile()
    return nc


def _get_nc(n_chunks=16, bsh=BSH):
    key = (n_chunks, bsh)
    if key not in _NC_CACHE:
        _NC_CACHE[key] = build(n_chunks, bsh)
    return _NC_CACHE[key]


def _consts():
    iota = np.broadcast_to(np.arange(T, dtype=F32_NP),
                           (128, T)).astype(BF16_NP)
    ident = np.eye(128, dtype=F32_NP)
    identb = np.eye(128, dtype=F32_NP).astype(BF16_NP)
    revjb = np.eye(128, dtype=F32_NP)[::-1].astype(BF16_NP)
    bones = np.zeros((128, 2), dtype=F32_NP)
    bones[0:T, 0] = 1.0
    bones[T:128, 1] = 1.0
    bsel = np.zeros((2, 128), dtype=F32_NP)
    bsel[0, 0:T] = 1.0
    bsel[1, T:128] = 1.0
    return iota, ident, identb, revjb, bones.astype(BF16_NP), \
        bsel.astype(BF16_NP)


def _shift_tags(tags_f):
    tq = np.empty_like(tags_f)
    tq[:, :-1] = tags_f[:, 1:]
    tq[:, -1] = -1.0
    return tq


def make_in_maps(emissions, start_transitions, end_transitions, transitions,
                 tags, ncores=NCORES):
    """Host prep: fold start/end into em, convert to bf16, shard."""
    em = np.asarray(emissions, dtype=F32_NP).copy()
    em[:, 0, :] += np.asarray(start_transitions, dtype=F32_NP)
    em[:, -1, :] += np.asarray(end_transitions, dtype=F32_NP)
    em_b = np.ascontiguousarray(em.astype(BF16_NP))
    tags_f = np.asarray(tags).astype(F32_NP).reshape(em.shape[0],
                                                     em.shape[1])
    tags_b = np.ascontiguousarray(tags_f)
    tagsq_b = np.ascontiguousarray(_shift_tags(tags_f))
    trans = np.asarray(transitions, dtype=F32_NP).reshape(T, T)
    iota, ident, identb, revjb, bones, bsel = _consts()
    bsh = em.shape[0] // ncores
    in_maps = []
    for cidx in range(ncores):
        sl = slice(cidx * bsh, (cidx + 1) * bsh)
        in_maps.append({
            "em": em_b[sl],
            "tagsf": tags_b[sl],
            "tagsq": tagsq_b[sl],
            "trans": trans,
            "bones": bones,
            "bsel": bsel,
            "iotat": iota,
            "ident": ident,
            "identb": identb,
            "revjb": revjb,
        })
    return in_maps


def kernel(emissions, start_transitions, end_transitions, transitions,
           tags, mask):
    """Full-input entry point; shards over 8 NeuronCores internally."""
    from concourse.bass_utils import run_bass_kernel_spmd

    emissions = np.asarray(emissions)
    assert emissions.shape == (B, S, T)
    assert (np.asarray(mask) != 0).all(), "kernel assumes all-ones mask"

    in_maps = make_in_maps(emissions, start_transitions, end_transitions,
                           transitions, tags)
    nc = _get_nc()
    res = run_bass_kernel_spmd(nc, in_maps, core_ids=list(range(NCORES)))

    num_total = 0.0
    den_total = 0.0
    for cidx in range(NCORES):
        r = res.results[cidx]
        num_total += float(r["misc"].sum())
        den_total += float(r["den"].sum())
    loss = -(num_total - den_total) / float(B)
    return np.float32(loss)
